# revision 1
# baseline (speedup 1.0000x reference)
"""Trainium2 Bass kernel for the 2-layer liquid-NN multistep recurrence.

Math (reference):
    for t in 0..49:
        h0 = 0.9*h0 + 0.1*tanh(h0 @ Wh0 + x_t @ Wu0 + b0)
        h1 = 0.9*h1 + 0.1*tanh(h1 @ Wh1 + h0 @ Wu1 + b1)
    out = h1 @ fc_w + fc_b

Kernel strategy:
  - Data parallel over 8 NeuronCores: batch 8192 -> 1024 rows/core.
  - State kept TRANSPOSED in SBUF: tiles are [128(h), 512(b)], so every
    matmul contracts over the partition dim with naturally-laid-out weights
    (lhsT = W[h, ho] slice, rhs = state tile).
  - Rescaled state g_t = h_t / 0.9^t turns the update into a single fused
    axpy per tile:  g += (0.1*0.9^-(t+1)) * tanh(0.9^t * psum + b)
    (tanh input scale+bias ride free on the ACT instruction; the axpy is one
    DVE scalar_tensor_tensor).  Wu1 is pre-scaled by 0.9 host-side so both
    accumulation terms of cell 1 share the 0.9^t scale.
  - Everything the PE touches is fp16: same 11-bit mantissa as fp32r, but
    half the weight-load (LDWEIGHTS) traffic so back-to-back matmuls run at
    the 512-cycle floor.  PSUM accumulation stays fp32; the DVE axpy
    reads the f32 tanh and updates the fp16 state in place.
  - x ships compact as [8t+f, b] fp16 (rows past 8T zero-padded host-side),
    DMA'd as four dense [128, b] tiles = 16 timesteps each.  The x_t @ Wu0
    term is a full K=128 matmul against one of 16 on-device-built weight
    tiles (Wu0 at rows 8j..8j+8, zeros elsewhere) -- narrow-K matmuls run
    at half rate on the PE, full-K ones at line rate.
  - t=0 is specialized (state starts at zero): the Wh*state matmuls are
    skipped and the state is written directly, so no state memsets and the
    first matmul issues as soon as the first x tile lands.
"""

import os
import sys

import numpy as np

for _p in ("/opt/trn_rl_repo",):
    if _p not in sys.path:
        sys.path.insert(0, _p)

import concourse.bass as bass
import concourse.tile as tile
from concourse import bacc, bass_utils, mybir

F32 = mybir.dt.float32
F16 = mybir.dt.float16
AF = mybir.ActivationFunctionType
ALU = mybir.AluOpType

NCORES = 8
B = 8192
BL = B // NCORES  # 1024
S = 50
F = 8
H = 512
P = 10
T = 50
DT = 0.1
DEC = 1.0 - DT
KT = H // 128  # 4 k/ho tiles
NH = 2  # batch halves of 512
NHW = BL // NH  # 512

TPX = 128 // F  # 16 timesteps per x tile
NXT = (T + TPX - 1) // TPX  # 4 x tiles
XROWS = NXT * 128  # 512 padded x rows


def build_program():
    nc = bacc.Bacc(
        "TRN2", target_bir_lowering=False, debug=False, num_devices=NCORES
    )
    xT_d = nc.dram_tensor("xTp", [XROWS, BL], F16, kind="ExternalInput").ap()
    wh0_d = nc.dram_tensor("Wh0", [H, H], F16, kind="ExternalInput").ap()
    wh1_d = nc.dram_tensor("Wh1", [H, H], F16, kind="ExternalInput").ap()
    wu1_d = nc.dram_tensor("Wu1s", [H, H], F16, kind="ExternalInput").ap()
    wu0_d = nc.dram_tensor("Wu0", [F, H], F16, kind="ExternalInput").ap()
    b0_d = nc.dram_tensor("b0m", [128, KT], F32, kind="ExternalInput").ap()
    b1_d = nc.dram_tensor("b1m", [128, KT], F32, kind="ExternalInput").ap()
    fc_d = nc.dram_tensor("fc_w", [H, P], F16, kind="ExternalInput").ap()
    fcb_d = nc.dram_tensor("fc_bm", [P, 1], F32, kind="ExternalInput").ap()
    out_d = nc.dram_tensor("outT", [P, BL], F32, kind="ExternalOutput").ap()

    from contextlib import ExitStack

    with tile.TileContext(nc) as tc, ExitStack() as ctx:
        const = ctx.enter_context(tc.tile_pool(name="const", bufs=1))
        tanh_pool = ctx.enter_context(tc.tile_pool(name="tanh", bufs=4))
        psum = ctx.enter_context(tc.tile_pool(name="psum", bufs=8, space="PSUM"))

        # ---- load weights / constants, in the order t=0 consumes them -----
        xt = []
        for c in range(NXT):
            t_ = const.tile([128, BL], F16, tag=f"xt_{c}")
            xt.append(t_)
        nc.sync.dma_start(xt[0][:], xT_d[0:128, :])

        # 16 padded Wu0 weight tiles: Wu0 at rows 8j..8j+8, zeros elsewhere.
        # Built on-device: GpSimd memset + an 8-row DMA from the tiny Wu0.
        # Only j=0 is DMA'd up front -- j=1..15 queue after the big weight
        # tiles so they don't delay the t=0/t=1 critical DMAs.
        wu0 = []
        for j in range(TPX):
            t_ = const.tile([128, H], F16, tag=f"wu0_{j}")
            nc.gpsimd.memset(t_[:], 0.0)
            wu0.append(t_)
        nc.sync.dma_start(wu0[0][0:F, :], wu0_d[:, :])
        b0m = const.tile([128, KT], F32, tag="b0m")
        nc.sync.dma_start(b0m[:], b0_d[:, :])

        wu1 = []
        for k in range(KT):
            t_ = const.tile([128, H], F16, tag=f"wu1_{k}")
            nc.sync.dma_start(t_[:], wu1_d[k * 128 : (k + 1) * 128, :])
            wu1.append(t_)
        b1m = const.tile([128, KT], F32, tag="b1m")
        nc.sync.dma_start(b1m[:], b1_d[:, :])

        wh0 = []
        wh1 = []
        for k in range(KT):
            t_ = const.tile([128, H], F16, tag=f"wh0_{k}")
            nc.sync.dma_start(t_[:], wh0_d[k * 128 : (k + 1) * 128, :])
            wh0.append(t_)
        for k in range(KT):
            t_ = const.tile([128, H], F16, tag=f"wh1_{k}")
            nc.sync.dma_start(t_[:], wh1_d[k * 128 : (k + 1) * 128, :])
            wh1.append(t_)

        for j in range(1, TPX):
            nc.sync.dma_start(wu0[j][F * j : F * j + F, :], wu0_d[:, :])
        for c in range(1, NXT):
            nc.sync.dma_start(xt[c][:], xT_d[c * 128 : (c + 1) * 128, :])

        fcw = []
        for k in range(KT):
            t_ = const.tile([128, P], F16, tag=f"fcw_{k}")
            nc.sync.dma_start(t_[:], fc_d[k * 128 : (k + 1) * 128, :])
            fcw.append(t_)
        fcb = const.tile([P, 1], F32, tag="fcb")
        nc.sync.dma_start(fcb[:], fcb_d[:, :])

        # ---- state tiles (separate tile per k-block per half: avoids false
        # cross-half dependencies).  No memsets: the specialized t=0 step
        # writes them before anything reads them. ---------------------------
        g0 = [[None] * NH for _ in range(KT)]
        g1 = [[None] * NH for _ in range(KT)]
        for k in range(KT):
            for h in range(NH):
                a = const.tile([128, NHW], F16, tag=f"g0_{k}_{h}")
                g0[k][h] = a
                a = const.tile([128, NHW], F16, tag=f"g1_{k}_{h}")
                g1[k][h] = a

        outT = const.tile([P, BL], F32, tag="outT")

        # ---- recurrence ----------------------------------------------------
        reps = int(os.environ.get("KERNEL_REPEAT", "1"))
        steps = [(t, t == 0 and r == 0) for r in range(reps) for t in range(T)]
        for t, first in steps:
            s_in = float(DEC**t)
            c_upd = float(DT * DEC ** -(t + 1))
            xc, xj = t // TPX, t % TPX
            for h in range(NH):
                # cell 0: z0 = Wh0^T g0 + Wu0p^T x~_t.  Phase A: all matmul
                # groups + tanh against the OLD state; phase B: all updates.
                t0s = []
                for m in range(KT):
                    ms = slice(m * 128, (m + 1) * 128)
                    pz = psum.tile([128, NHW], F32, tag="pz")
                    if not first:
                        for k in range(KT):
                            nc.tensor.matmul(
                                pz[:],
                                wh0[k][:, ms],
                                g0[k][h][:],
                                start=(k == 0),
                                stop=False,
                            )
                    nc.tensor.matmul(
                        pz[:],
                        wu0[xj][:, ms],
                        xt[xc][:, h * NHW : (h + 1) * NHW],
                        start=first,
                        stop=True,
                    )
                    t0 = tanh_pool.tile([128, NHW], F32, tag="t0")
                    nc.scalar.activation(
                        t0[:], pz[:], AF.Tanh, bias=b0m[:, m : m + 1], scale=s_in
                    )
                    t0s.append(t0)
                for m in range(KT):
                    # g0[m] += c_upd * t0   (fused axpy)
                    if first:
                        nc.vector.tensor_scalar_mul(g0[m][h][:], t0s[m][:], c_upd)
                    else:
                        nc.vector.scalar_tensor_tensor(
                            g0[m][h][:],
                            t0s[m][:],
                            c_upd,
                            g0[m][h][:],
                            ALU.mult,
                            ALU.add,
                        )
                # cell 1: z1 = Wh1^T g1 + (0.9*Wu1)^T g0'
                t1s = []
                for m in range(KT):
                    ms = slice(m * 128, (m + 1) * 128)
                    pz = psum.tile([128, NHW], F32, tag="pz")
                    if not first:
                        for k in range(KT):
                            nc.tensor.matmul(
                                pz[:],
                                wh1[k][:, ms],
                                g1[k][h][:],
                                start=(k == 0),
                                stop=False,
                            )
                    for k in range(KT):
                        nc.tensor.matmul(
                            pz[:],
                            wu1[k][:, ms],
                            g0[k][h][:],
                            start=(first and k == 0),
                            stop=(k == KT - 1),
                        )
                    t1 = tanh_pool.tile([128, NHW], F32, tag="t1")
                    nc.scalar.activation(
                        t1[:], pz[:], AF.Tanh, bias=b1m[:, m : m + 1], scale=s_in
                    )
                    t1s.append(t1)
                for m in range(KT):
                    if first:
                        nc.vector.tensor_scalar_mul(g1[m][h][:], t1s[m][:], c_upd)
                    else:
                        nc.vector.scalar_tensor_tensor(
                            g1[m][h][:],
                            t1s[m][:],
                            c_upd,
                            g1[m][h][:],
                            ALU.mult,
                            ALU.add,
                        )

        # ---- output head: outT = 0.9^T * (fc_w^T g1) + fc_b ---------------
        for h in range(NH):
            po = psum.tile([128, NHW], F32, tag="pz")
            for k in range(KT):
                nc.tensor.matmul(
                    po[0:P, :],
                    fcw[k][:, 0:P],
                    g1[k][h][:],
                    start=(k == 0),
                    stop=(k == KT - 1),
                )
            nc.scalar.activation(
                outT[0:P, h * NHW : (h + 1) * NHW],
                po[0:P, :],
                AF.Identity,
                bias=fcb[:, 0:1],
                scale=float(DEC**T),
            )
            nc.sync.dma_start(
                out_d[:, h * NHW : (h + 1) * NHW],
                outT[0:P, h * NHW : (h + 1) * NHW],
            )

    nc.compile()
    return nc


_NC_CACHE = None


def _get_program():
    global _NC_CACHE
    if _NC_CACHE is None:
        _NC_CACHE = build_program()
    return _NC_CACHE


def _prep_inputs(x, Wh0, Wu0, b0, Wh1, Wu1, b1, fc_w, fc_b):
    """Host-side prep: shard + transpose/rescale/pad x, pre-scale Wu1."""
    dec_inv = (DEC ** -np.arange(T, dtype=np.float64)).astype(np.float32)
    # [B, S, F] -> take T steps, scale by 0.9^-t, -> [T, F, B] fp16,
    # flattened to rows 8t+f and zero-padded to XROWS.
    xs = (np.asarray(x[:, :T, :], np.float32) * dec_inv[None, :, None]).astype(
        np.float16
    )
    xp = np.zeros((XROWS, B), np.float16)
    xp[: T * F] = xs.transpose(1, 2, 0).reshape(T * F, B)

    shared = {
        "Wh0": np.asarray(Wh0, np.float32).astype(np.float16),
        "Wh1": np.asarray(Wh1, np.float32).astype(np.float16),
        "Wu1s": (np.asarray(Wu1, np.float32) * np.float32(DEC)).astype(np.float16),
        "Wu0": np.asarray(Wu0, np.float32).astype(np.float16),
        "b0m": np.ascontiguousarray(np.asarray(b0, np.float32).reshape(KT, 128).T),
        "b1m": np.ascontiguousarray(np.asarray(b1, np.float32).reshape(KT, 128).T),
        "fc_w": np.asarray(fc_w, np.float32).astype(np.float16),
        "fc_bm": np.ascontiguousarray(np.asarray(fc_b, np.float32).reshape(P, 1)),
    }
    in_maps = []
    for c in range(NCORES):
        m = dict(shared)
        m["xTp"] = np.ascontiguousarray(xp[:, c * BL : (c + 1) * BL])
        in_maps.append(m)
    return in_maps


def run(inputs, trace=False, **kw):
    nc = _get_program()
    in_maps = _prep_inputs(**inputs)
    res = bass_utils.run_bass_kernel_spmd(
        nc, in_maps, core_ids=list(range(NCORES)), trace=trace, **kw
    )
    out = np.empty((B, P), np.float32)
    for c in range(NCORES):
        out[c * BL : (c + 1) * BL, :] = res.results[c]["outT"].T
    return out, res


def kernel(**inputs):
    out, _ = run(inputs, trace=False)
    return out


if __name__ == "__main__":
    print("smoke test: building program...")
    nc = _get_program()
    print("built ok")



# revision 15
# speedup vs baseline: 1.8186x; 1.8186x over previous
"""Trainium2 Bass kernel for the 2-layer liquid-NN multistep recurrence.

Math (reference):
    for t in 0..49:
        h0 = 0.9*h0 + 0.1*tanh(h0 @ Wh0 + x_t @ Wu0 + b0)
        h1 = 0.9*h1 + 0.1*tanh(h1 @ Wh1 + h0 @ Wu1 + b1)
    out = h1 @ fc_w + fc_b

Kernel strategy (fp8 DoubleRow edition):
  - Data parallel over 8 NeuronCores: batch 8192 -> 1024 rows/core.
  - State stored as s = h/0.1 (so the update is a single DVE op
    s' = 0.9*s + tanh(...) with the 0.1 folded into the recurrent weights).
    fp16 master state [128(h), 2, 512(b)] per (cell, m-tile); an fp8e4 feed
    copy per step is what the PE contracts against.
  - The three H x H matmuls per step run in fp8 DoubleRow mode: weights and
    state feed are fp8e4, each matmul contracts TWO 128-row k-tiles (planes),
    halving PE instruction count vs fp16.  Weights are pre-scaled by
    ALPHA=8192 so fp8's normal range is used; the ACT tanh un-scales.
  - fp8 weight quantization error is the dominant error term, so the weights
    are 4-phase DITHERED: four fp8 versions per matrix whose rounding errors
    time-average to ~ULP/8; step t uses version t%4.  The recurrence's EMA
    (0.9 decay) suppresses the alternating components ~13x, leaving only the
    small DC part.  Simulated end-to-end rel err ~9e-3 (vs 2e-2 budget).
  - The x @ Wu0 term stays fp16 (x quantization alone would eat the error
    budget): x ships compact as [8t+f, b] fp16 rows, and is contracted with
    one of 16 on-device-built padded Wu0 tiles (ALPHA*Wu0 rows at 8j..8j+8,
    zeros elsewhere) as a full-K=128 fp16 matmul accumulating into the same
    PSUM group as the DoubleRow pair matmuls.
  - Per step per (cell, m-tile): matmuls -> one ACT tanh [128, 1024] (bias +
    1/ALPHA scale ride free) -> one DVE axpy (0.9*s + T, fp16, 2x mode) ->
    one DVE copy master->fp8 feed.  ACT ~9.2us, DVE ~11us, PE ~13.3us per
    step: PE-bound.
  - t=0 specialized: state is zero, so the Wh matmuls are skipped and ACT
    writes tanh directly into the master (no axpy).
"""

import os
import sys

import numpy as np

for _p in ("/opt/trn_rl_repo",):
    if _p not in sys.path:
        sys.path.insert(0, _p)

import concourse.bass as bass
import concourse.tile as tile
from concourse import bacc, bass_utils, mybir

F32 = mybir.dt.float32
F16 = mybir.dt.float16
F8 = mybir.dt.float8e4
AF = mybir.ActivationFunctionType
ALU = mybir.AluOpType
PM = mybir.MatmulPerfMode

NCORES = 8
B = 8192
BL = B // NCORES  # 1024
S = 50
F = 8
H = 512
P = 10
T = 50
DT = 0.1
KT = H // 128  # 4 k/m tiles
NPAIR = KT // 2  # 2 DoubleRow k-tile pairs
NPHASE = 4  # weight dither phases
ALPHA = 8192.0
NH = 2  # batch halves of 512
NHW = BL // NH  # 512

TPX = 128 // F  # 16 timesteps per x tile
NXT = (T + TPX - 1) // TPX  # 4 x tiles
XROWS = NXT * 128  # 512 padded x rows


def build_program(t_steps=T, dump_state=False, dump_z=False):
    nc = bacc.Bacc(
        "TRN2", target_bir_lowering=False, debug=False, num_devices=NCORES
    )
    xT_d = nc.dram_tensor("xTp", [XROWS, BL], F16, kind="ExternalInput").ap()
    wh0_d = nc.dram_tensor("wh0v", [NPHASE * H, H], F8, kind="ExternalInput").ap()
    wh1_d = nc.dram_tensor("wh1v", [NPHASE * H, H], F8, kind="ExternalInput").ap()
    wu1_d = nc.dram_tensor("wu1v", [NPHASE * H, H], F8, kind="ExternalInput").ap()
    wu0_d = nc.dram_tensor("wu0s", [F, H], F16, kind="ExternalInput").ap()
    b0_d = nc.dram_tensor("b0m", [128, KT], F32, kind="ExternalInput").ap()
    b1_d = nc.dram_tensor("b1m", [128, KT], F32, kind="ExternalInput").ap()
    fc_d = nc.dram_tensor("fc_w", [H, P], F16, kind="ExternalInput").ap()
    fcb_d = nc.dram_tensor("fc_bm", [P, 1], F32, kind="ExternalInput").ap()
    out_d = nc.dram_tensor("outT", [P, BL], F32, kind="ExternalOutput").ap()
    dump_d = None
    if dump_state:
        dump_d = [
            nc.dram_tensor(f"dump{c}", [H, BL], F16, kind="ExternalOutput").ap()
            for c in range(2)
        ]
    dumpz_d = None
    if dump_z:
        dumpz_d = nc.dram_tensor("dumpz", [H, BL], F32, kind="ExternalOutput").ap()

    from contextlib import ExitStack

    with tile.TileContext(nc) as tc, ExitStack() as ctx:
        const = ctx.enter_context(tc.tile_pool(name="const", bufs=1))
        tanh_pool = ctx.enter_context(tc.tile_pool(name="tanh", bufs=8))
        psum = ctx.enter_context(tc.tile_pool(name="psum", bufs=4, space="PSUM"))

        # ---- load constants, in the order the early steps consume them ----
        xt = [const.tile([128, BL], F16, tag=f"xt_{c}", name=f"xt_{c}") for c in range(NXT)]
        # t=0 needs only rows 0..8 of the first x tile; split so the first
        # matmul can issue after a ~16KB DMA instead of 256KB.
        nc.sync.dma_start(xt[0][0:F, :], xT_d[0:F, :])
        nc.sync.dma_start(xt[0][F:128, :], xT_d[F:128, :])

        # 16 padded Wu0 tiles (ALPHA*Wu0 at rows 8j..8j+8, zeros elsewhere)
        wu0 = []
        for j in range(TPX):
            t_ = const.tile([128, H], F16, tag=f"wu0_{j}")
            nc.gpsimd.memset(t_[:], 0.0)
            wu0.append(t_)
        nc.sync.dma_start(wu0[0][0:F, :], wu0_d[:, :])
        b0m = const.tile([128, KT], F32, tag="b0m")
        nc.sync.dma_start(b0m[:], b0_d[:, :])
        b1m = const.tile([128, KT], F32, tag="b1m")
        nc.sync.dma_start(b1m[:], b1_d[:, :])

        # weight tiles: [128, 2, 512] fp8, plane i = k-tile 2j+i (rows), all
        # 512 output cols.  w[X][v][j].
        def wtiles(name, dram):
            out = []
            for v in range(NPHASE):
                row = []
                for j in range(NPAIR):
                    t_ = const.tile([128, 2, H], F8, tag=f"{name}_{v}_{j}", name=f"{name}_{v}_{j}")
                    row.append(t_)
                out.append(row)
            return out

        wu1 = wtiles("wu1", wu1_d)
        wh0 = wtiles("wh0", wh0_d)
        wh1 = wtiles("wh1", wh1_d)

        def load_w(tiles, dram, v, j):
            for i in range(2):
                r0 = v * H + (2 * j + i) * 128
                nc.sync.dma_start(tiles[v][j][:, i, :], dram[r0 : r0 + 128, :])

        # Weight DMA order follows first-use: t=0 needs only wu1@v0 (the
        # first step skips the Wh matmuls); t=1..3 need the v1..v3 sets;
        # Wh@v0 isn't touched until t=4.
        for j in range(NPAIR):
            load_w(wu1, wu1_d, 0, j)
        for j in range(1, 3):
            nc.sync.dma_start(wu0[j][F * j : F * j + F, :], wu0_d[:, :])
        for v in range(1, NPHASE):
            for j in range(NPAIR):
                load_w(wh0, wh0_d, v, j)
                load_w(wh1, wh1_d, v, j)
                load_w(wu1, wu1_d, v, j)
        for j in range(NPAIR):
            load_w(wh0, wh0_d, 0, j)
            load_w(wh1, wh1_d, 0, j)
        for j in range(3, TPX):
            nc.sync.dma_start(wu0[j][F * j : F * j + F, :], wu0_d[:, :])
        for c in range(1, NXT):
            nc.sync.dma_start(xt[c][:], xT_d[c * 128 : (c + 1) * 128, :])

        fcw = []
        for k in range(KT):
            t_ = const.tile([128, P], F16, tag=f"fcw_{k}")
            nc.sync.dma_start(t_[:], fc_d[k * 128 : (k + 1) * 128, :])
            fcw.append(t_)
        fcb = const.tile([P, 1], F32, tag="fcb")
        nc.sync.dma_start(fcb[:], fcb_d[:, :])

        # ---- state ---------------------------------------------------------
        # master[c][m]: fp16 [128, 2(half), 512]  (s = h/0.1)
        # feed[c][j]:   fp8  [128, 2(half), 2(plane i), 512]
        master = [
            [
                const.tile([128, NH, NHW], F16, tag=f"ms{c}_{m}", name=f"ms{c}_{m}")
                for m in range(KT)
            ]
            for c in range(2)
        ]
        feed = [
            [
                const.tile([128, NH, 2, NHW], F8, tag=f"fd{c}_{j}", name=f"fd{c}_{j}")
                for j in range(NPAIR)
            ]
            for c in range(2)
        ]

        outT = const.tile([P, BL], F32, tag="outT")

        inv_a = float(1.0 / ALPHA)

        def cell_phase(c, t, first, wh, wu_feed_c, bm, xc=None, xj=None):
            """One cell, one step, two-phase: (A) all matmul groups + tanh
            against the OLD feed, (B) all axpys, (C) all feed copies.  The
            copies come last so every m-tile's matmuls read the pre-update
            state (Jacobi, matching the reference), and so the per-m PE
            groups run back-to-back without chaining through the DVE."""
            v = t % NPHASE
            t0s = []
            for m in range(KT):
                ms = slice(m * 128, (m + 1) * 128)
                pz = psum.tile([128, NH, NHW], F32, tag="pz")
                for h in range(NH):
                    started = False
                    if not first:
                        for j in range(NPAIR):
                            nc.tensor.matmul(
                                pz[:, h, :],
                                wh[v][j][:, :, ms],
                                feed[c][j][:, h, :, :],
                                start=not started,
                                stop=False,
                                perf_mode=PM.DoubleRow,
                            )
                            started = True
                    if c == 0:
                        # x-term: fp16 full-K matmul, closes the group
                        nc.tensor.matmul(
                            pz[:, h, :],
                            wu0[xj][:, ms],
                            xt[xc][:, h * NHW : (h + 1) * NHW],
                            start=not started,
                            stop=True,
                        )
                    else:
                        # u-term: DoubleRow against the *other* cell's new
                        # feed.  Emitted AFTER the Wh pairs so the group's
                        # head can issue while the copies land.
                        for j in range(NPAIR):
                            nc.tensor.matmul(
                                pz[:, h, :],
                                wu_feed_c[1][v][j][:, :, ms],
                                feed[wu_feed_c[0]][j][:, h, :, :],
                                start=not started,
                                stop=(j == NPAIR - 1),
                                perf_mode=PM.DoubleRow,
                            )
                            started = True
                if first:
                    nc.scalar.activation(
                        master[c][m][:],
                        pz[:],
                        AF.Tanh,
                        bias=bm[:, m : m + 1],
                        scale=inv_a,
                    )
                    t0s.append(None)
                else:
                    t0 = tanh_pool.tile([128, NH, NHW], F32, tag="t0")
                    nc.scalar.activation(
                        t0[:], pz[:], AF.Tanh, bias=bm[:, m : m + 1],
                        scale=inv_a,
                    )
                    if dump_z and c == 0 and t == t_steps - 1:
                        for h in range(NH):
                            nc.sync.dma_start(
                                dumpz_d[m * 128 : (m + 1) * 128,
                                        h * NHW : (h + 1) * NHW],
                                t0[:, h, :],
                            )
                    t0s.append(t0)
            if not first:
                for m in range(KT):
                    # s' = 0.9*s + T (DVE, fp16 in/out)
                    nc.vector.scalar_tensor_tensor(
                        master[c][m][:],
                        master[c][m][:],
                        float(1.0 - DT),
                        t0s[m][:],
                        ALU.mult,
                        ALU.add,
                    )
            for m in range(KT):
                for h in range(NH):
                    # fp8 feed copy: plane m%2 of pair m//2, half h
                    nc.vector.tensor_copy(
                        feed[c][m // 2][:, h, m % 2, :],
                        master[c][m][:, h, :],
                    )

        reps = int(os.environ.get("KERNEL_REPEAT", "1"))
        steps = [
            (t, t == 0 and r == 0) for r in range(reps) for t in range(t_steps)
        ]
        for t, first in steps:
            xc, xj = t // TPX, t % TPX
            cell_phase(0, t, first, wh0, None, b0m, xc=xc, xj=xj)
            cell_phase(1, t, first, wh1, (0, wu1), b1m)

        if dump_state:
            for c in range(2):
                for m in range(KT):
                    for h in range(NH):
                        nc.sync.dma_start(
                            dump_d[c][m * 128 : (m + 1) * 128,
                                      h * NHW : (h + 1) * NHW],
                            master[c][m][:, h, :],
                        )

        # ---- output head: outT = 0.1 * (fc_w^T s1) + fc_b -----------------
        po = psum.tile([128, NH, NHW], F32, tag="pz")
        for h in range(NH):
            for k in range(KT):
                nc.tensor.matmul(
                    po[0:P, h, :],
                    fcw[k][:, 0:P],
                    master[1][k][:, h, :],
                    start=(k == 0),
                    stop=(k == KT - 1),
                )
            nc.scalar.activation(
                outT[0:P, h * NHW : (h + 1) * NHW],
                po[0:P, h, :],
                AF.Identity,
                bias=fcb[:, 0:1],
                scale=float(DT),
            )
            nc.sync.dma_start(
                out_d[:, h * NHW : (h + 1) * NHW],
                outT[0:P, h * NHW : (h + 1) * NHW],
            )

    nc.compile()
    return nc


_NC_CACHE = None


def _get_program():
    global _NC_CACHE
    if _NC_CACHE is None:
        _NC_CACHE = build_program()
    return _NC_CACHE


# ---- host-side weight dithering -------------------------------------------
import ml_dtypes

_allv = np.arange(256, dtype=np.uint8).view(ml_dtypes.float8_e4m3).astype(np.float32)
_GRID = np.unique(_allv[np.isfinite(_allv)])


def _dither_versions(W, alpha, nphase):
    """nphase fp8 versions of alpha*W whose rounding time-averages toward
    alpha*W (Bresenham schedule per element)."""
    Ws = np.clip(alpha * W, -240.0, 240.0)
    hi = _GRID[np.clip(np.searchsorted(_GRID, Ws, side="left"), 0, len(_GRID) - 1)]
    lo = _GRID[np.clip(np.searchsorted(_GRID, Ws, side="right") - 1, 0, len(_GRID) - 1)]
    U = hi - lo
    p = np.where(U > 0, (Ws - lo) / np.where(U > 0, U, 1.0), 0.0)
    o = 0.499999
    vers = []
    for v in range(nphase):
        c = np.floor((v + 1) * p + o) - np.floor(v * p + o)
        vers.append((lo + c * U).astype(ml_dtypes.float8_e4m3))
    return np.concatenate(vers, axis=0)  # [nphase*H, H]


def _prep_inputs(x, Wh0, Wu0, b0, Wh1, Wu1, b1, fc_w, fc_b):
    """Host prep: shard + pack x, dither-quantize the recurrent weights."""
    xs = np.asarray(x[:, :T, :], np.float32).astype(np.float16)
    xp = np.zeros((XROWS, B), np.float16)
    xp[: T * F] = xs.transpose(1, 2, 0).reshape(T * F, B)

    shared = {
        "wh0v": _dither_versions(DT * np.asarray(Wh0, np.float64), ALPHA, NPHASE),
        "wh1v": _dither_versions(DT * np.asarray(Wh1, np.float64), ALPHA, NPHASE),
        "wu1v": _dither_versions(DT * np.asarray(Wu1, np.float64), ALPHA, NPHASE),
        "wu0s": (ALPHA * np.asarray(Wu0, np.float32)).astype(np.float16),
        "b0m": np.ascontiguousarray(np.asarray(b0, np.float32).reshape(KT, 128).T),
        "b1m": np.ascontiguousarray(np.asarray(b1, np.float32).reshape(KT, 128).T),
        "fc_w": np.asarray(fc_w, np.float32).astype(np.float16),
        "fc_bm": np.ascontiguousarray(np.asarray(fc_b, np.float32).reshape(P, 1)),
    }
    in_maps = []
    for c in range(NCORES):
        m = dict(shared)
        m["xTp"] = np.ascontiguousarray(xp[:, c * BL : (c + 1) * BL])
        in_maps.append(m)
    return in_maps


def run(inputs, trace=False, **kw):
    nc = _get_program()
    in_maps = _prep_inputs(**inputs)
    res = bass_utils.run_bass_kernel_spmd(
        nc, in_maps, core_ids=list(range(NCORES)), trace=trace, **kw
    )
    out = np.empty((B, P), np.float32)
    for c in range(NCORES):
        out[c * BL : (c + 1) * BL, :] = res.results[c]["outT"].T
    return out, res


def kernel(**inputs):
    out, _ = run(inputs, trace=False)
    return out


if __name__ == "__main__":
    print("smoke test: building program...")
    nc = _get_program()
    print("built ok")
